# revision 1
# baseline (speedup 1.0000x reference)
"""BiDecoder edge kernel for Trainium2, 8-core SPMD.

out[e, r] = (u[edge_user[e]] @ W[r].T + b[r]) . i[edge_item[e]]
          = sum_j i_row[j] * ( sum_k W[r,j,k] u_row[k] + b[r,j] )

Distribution: edges sharded by contiguous user ranges across 8 cores; each
core receives its own user-table slice (u_shard) plus the full item table.

Per-core algorithm (all data-dependent access via int16 dma_gather):
  - host sorts the shard by (item_window, user) and packs 128-edge tiles with
    <= NQ unique users each;
  - item rows are bulk-gathered per edge (window-relative int16 idx);
  - unique user rows are gathered once per tile (shard-relative idx);
  - per 8-tile group, PE computes C[j,(r,q)] = W_r @ U_uniq^T + b_r (bias
    folded via a K=1 accumulating matmul);
  - per tile, PE computes G[e,(r,q)] = I_tile^T . C  (I^T stationary);
  - DVE selects q = pos(e) with a batched one-hot multiply + reduce;
  - PE transposes output batches so the DRAM write is dense.
"""
import numpy as np
from contextlib import ExitStack

import concourse.bacc as bacc
import concourse.bass as bass
import concourse.tile as tile
from concourse import mybir
from concourse.bass_utils import run_bass_kernel_spmd
from concourse.masks import make_identity

F32 = mybir.dt.float32
I16 = mybir.dt.int16

N_CORES = 8
D = 64
R = 5
NQ = 16            # max unique users per 128-edge tile
CGRP = 8           # tiles per C-group (CGRP*NQ = 128 unique rows)
TBATCH = 8         # tiles per select/output batch
NT_ALIGN = 8       # lcm(CGRP, TBATCH)
BLK_TILES = 64     # tiles per bulk dma_gather block (8192 edges)
UBLK = 2048        # unique rows per uniq dma_gather block
IWIN = 32768       # item-window size (int16 index limit)


# ----------------------------------------------------------------- host prep

def _prep_core(users, items, eidx):
    """Tile-pack one core's edges. Edges are sorted here by
    (item_window, user). Returns dict of per-tile arrays."""
    iwin_all = items // IWIN
    order = np.lexsort((users, iwin_all))
    users = users[order]; items = items[order]; eidx = eidx[order]
    iwin_all = iwin_all[order]

    n = len(users)
    newgrp = np.ones(n, dtype=bool)
    if n > 1:
        newgrp[1:] = (users[1:] != users[:-1]) | (iwin_all[1:] != iwin_all[:-1])
    grp_starts = np.nonzero(newgrp)[0]
    grp_ends = np.append(grp_starts[1:], n)

    tile_win, slot_item, slot_eidx, slot_pos, tile_uniq = [], [], [], [], []
    ci, ce, cp, cuq = [], [], [], []
    cw = -1

    def flush():
        nonlocal ci, ce, cp, cuq
        if not ci:
            return
        pi = ci[0]
        while len(ci) < 128:
            ci.append(pi); ce.append(-1); cp.append(0)
        uq = list(cuq)
        while len(uq) < NQ:
            uq.append(uq[0])
        tile_win.append(cw)
        slot_item.append(ci); slot_eidx.append(ce)
        slot_pos.append(cp); tile_uniq.append(uq)
        ci, ce, cp, cuq = [], [], [], []

    for gs, ge in zip(grp_starts, grp_ends):
        w = int(iwin_all[gs]); u = int(users[gs])
        pos = int(gs)
        while pos < ge:
            if ci and (cw != w or len(ci) >= 128 or
                       (u not in cuq and len(cuq) >= NQ)):
                flush()
            cw = w
            if u not in cuq:
                cuq.append(u)
            p = cuq.index(u)
            take = min(int(ge) - pos, 128 - len(ci))
            ci.extend(items[pos:pos + take].tolist())
            ce.extend(eidx[pos:pos + take].tolist())
            cp.extend([p] * take)
            pos += take
    flush()

    return {
        "tile_win": np.array(tile_win, dtype=np.int64),
        "slot_item": np.array(slot_item, dtype=np.int64),
        "slot_eidx": np.array(slot_eidx, dtype=np.int64),
        "slot_pos": np.array(slot_pos, dtype=np.int64),
        "tile_uniq": np.array(tile_uniq, dtype=np.int64),
    }


def _wrap16(idx_i16):
    """[n] int16 (n%16==0) -> [128, n//16]: idx j -> row j%16, col j//16,
    16-row block replicated to 128 partitions."""
    n = len(idx_i16)
    w = idx_i16.reshape(n // 16, 16).T
    return np.ascontiguousarray(np.tile(w, (8, 1)))


def host_prep(u_features, i_features, edge_user, edge_item):
    E = len(edge_user)
    NU = u_features.shape[0]
    NI = i_features.shape[0]
    n_iwin = (NI + IWIN - 1) // IWIN

    eu = np.asarray(edge_user, dtype=np.int64)
    ei = np.asarray(edge_item, dtype=np.int64)

    # shard by contiguous user ranges with ~equal edge counts
    order_u = np.argsort(eu, kind="stable")
    eus = eu[order_u]
    bounds = [0]
    for c in range(1, N_CORES):
        t = (E * c) // N_CORES
        while 0 < t < E and eus[t] == eus[t - 1]:
            t += 1
        bounds.append(min(t, E))
    bounds.append(E)

    cores = []
    for c in range(N_CORES):
        sl = order_u[bounds[c]:bounds[c + 1]]
        cores.append(_prep_core(eu[sl], ei[sl], sl.copy()))

    ushard_start, ushard_len = [], []
    for c in range(N_CORES):
        uq = cores[c]["tile_uniq"]
        lo = int(uq.min()) if uq.size else 0
        hi = int(uq.max()) + 1 if uq.size else 1
        ushard_start.append(lo); ushard_len.append(hi - lo)
    MAXU = max(ushard_len)
    assert MAXU <= 32768, f"user shard {MAXU} exceeds int16 range"

    # equalize per-item-window tile counts across cores
    sec_tiles = np.zeros((N_CORES, n_iwin), dtype=np.int64)
    for c in range(N_CORES):
        for w in range(n_iwin):
            sec_tiles[c, w] = int((cores[c]["tile_win"] == w).sum())
    sec_pad = ((sec_tiles.max(axis=0) + CGRP - 1) // CGRP) * CGRP
    sec_pad = np.maximum(sec_pad, CGRP)
    while int(sec_pad.sum()) % NT_ALIGN != 0:
        sec_pad[-1] += CGRP
    NT = int(sec_pad.sum())
    n_bulk_blocks = [(int(s) + BLK_TILES - 1) // BLK_TILES for s in sec_pad]
    sec_blk_tiles = [n_bulk_blocks[w] * BLK_TILES for w in range(n_iwin)]
    NT_U = ((NT * NQ + UBLK - 1) // UBLK) * UBLK // NQ  # tiles covered by ublks

    in_maps, reasm = [], []
    for c in range(N_CORES):
        d = cores[c]
        slot_item_p = np.zeros((NT, 128), dtype=np.int64)
        slot_pos_p = np.zeros((NT, 128), dtype=np.int64)
        tile_uniq_p = np.zeros((NT, NQ), dtype=np.int64)
        slot_eidx_p = np.full((NT, 128), -1, dtype=np.int64)
        t_out = 0
        for w in range(n_iwin):
            sel = np.nonzero(d["tile_win"] == w)[0]
            k = len(sel); spw = int(sec_pad[w])
            if k:
                slot_item_p[t_out:t_out + k] = d["slot_item"][sel]
                slot_pos_p[t_out:t_out + k] = d["slot_pos"][sel]
                tile_uniq_p[t_out:t_out + k] = d["tile_uniq"][sel]
                slot_eidx_p[t_out:t_out + k] = d["slot_eidx"][sel]
            slot_item_p[t_out + k:t_out + spw] = w * IWIN
            tile_uniq_p[t_out + k:t_out + spw] = ushard_start[c]
            t_out += spw
        assert t_out == NT

        bulk_parts = []
        t0 = 0
        for w in range(n_iwin):
            spw = int(sec_pad[w])
            rel = (slot_item_p[t0:t0 + spw] - w * IWIN).astype(np.int16).ravel()
            pad = np.zeros(128 * (sec_blk_tiles[w] - spw), dtype=np.int16)
            bulk_parts.append(np.concatenate([rel, pad]))
            t0 += spw
        bulk_wrapped = _wrap16(np.concatenate(bulk_parts))

        uq = np.zeros((NT_U, NQ), dtype=np.int64)
        uq[:NT] = tile_uniq_p - ushard_start[c]
        uniq_wrapped = _wrap16(uq.ravel().astype(np.int16))

        pos_f32 = np.ascontiguousarray(slot_pos_p.T.astype(np.float32))

        ush = np.zeros((MAXU, D), dtype=np.float32)
        n_avail = min(MAXU, NU - ushard_start[c])
        ush[:n_avail] = u_features[ushard_start[c]:ushard_start[c] + n_avail]

        in_maps.append({
            "u_shard": ush,
            "i_tab": np.ascontiguousarray(i_features, dtype=np.float32),
            "bulk_idx": bulk_wrapped,
            "uniq_idx": uniq_wrapped,
            "pos": pos_f32,
        })
        reasm.append(slot_eidx_p)

    meta = {
        "NT": NT, "NT_U": NT_U, "n_iwin": n_iwin,
        "n_bulk_blocks": n_bulk_blocks, "sec_blk_tiles": sec_blk_tiles,
        "sec_pad": [int(x) for x in sec_pad], "MAXU": MAXU, "NI": NI, "E": E,
    }
    return in_maps, reasm, meta


# ------------------------------------------------------------ device program

def _bc_mid(ap, size, axis):
    """Insert a stride-0 dim of `size` at free-dim position `axis` (counting
    the partition dim as 0)."""
    lst = [list(x) for x in ap.ap]
    lst.insert(axis, [0, size])
    return bass.AP(ap.tensor, ap.offset, [tuple(x) for x in lst])


def build_program(meta):
    NT, NT_U = meta["NT"], meta["NT_U"]
    n_iwin = meta["n_iwin"]
    MAXU, NI = meta["MAXU"], meta["NI"]
    n_bulk_blocks = meta["n_bulk_blocks"]
    sec_pad = meta["sec_pad"]
    NBULKC = sum(meta["sec_blk_tiles"]) * 8   # bulk_idx columns
    n_ublk = NT_U * NQ // UBLK

    nc = bacc.Bacc("TRN2", target_bir_lowering=False, debug=False,
                   num_devices=N_CORES, num_swdge_queues=4)

    u_shard = nc.dram_tensor("u_shard", [MAXU, D], F32, kind="ExternalInput").ap()
    i_tab = nc.dram_tensor("i_tab", [NI, D], F32, kind="ExternalInput").ap()
    bulk_idx = nc.dram_tensor("bulk_idx", [128, NBULKC], I16,
                              kind="ExternalInput").ap()
    uniq_idx = nc.dram_tensor("uniq_idx", [128, NT_U * NQ // 16], I16,
                              kind="ExternalInput").ap()
    pos_in = nc.dram_tensor("pos", [128, NT], F32, kind="ExternalInput").ap()
    wb = nc.dram_tensor("wb", [65, R, 64], F32, kind="ExternalInput").ap()
    bvec = nc.dram_tensor("bvec", [1, R, 64], F32, kind="ExternalInput").ap()
    ones_in = nc.dram_tensor("ones_in", [1, 128], F32, kind="ExternalInput").ap()
    iota_in = nc.dram_tensor("iota16", [128, NQ], F32, kind="ExternalInput").ap()
    out_dev = nc.dram_tensor("out_dev", [NT * R, 128], F32,
                             kind="ExternalOutput").ap()

    with tile.TileContext(nc) as tc, ExitStack() as ctx:
        cpool = ctx.enter_context(tc.tile_pool(name="const", bufs=1))
        bulkpool = ctx.enter_context(tc.tile_pool(name="bulk", bufs=3))
        itpool = ctx.enter_context(tc.tile_pool(name="it", bufs=6))
        csbpool = ctx.enter_context(tc.tile_pool(name="csb", bufs=3))
        selpool = ctx.enter_context(tc.tile_pool(name="sel", bufs=3))
        outpool = ctx.enter_context(tc.tile_pool(name="outb", bufs=2))
        psg = ctx.enter_context(tc.tile_pool(name="psg", bufs=2, space="PSUM"))
        psc = ctx.enter_context(tc.tile_pool(name="psc", bufs=1, space="PSUM"))
        pst = ctx.enter_context(tc.tile_pool(name="pst", bufs=2, space="PSUM"))

        ident = cpool.tile([128, 128], F32)
        make_identity(nc, ident[:])
        wb_sb = cpool.tile([65, R, 64], F32)
        nc.sync.dma_start(wb_sb[:], wb[:])
        b_sb = cpool.tile([1, R, 64], F32)
        nc.sync.dma_start(b_sb[:], bvec[:])
        ones_sb = cpool.tile([1, 128], F32)
        nc.sync.dma_start(ones_sb[:], ones_in[:])
        iota_sb = cpool.tile([128, NQ], F32)
        nc.sync.dma_start(iota_sb[:], iota_in[:])
        pos_sb = cpool.tile([128, NT], F32)
        nc.sync.dma_start(pos_sb[:], pos_in[:])
        bulk_idx_sb = cpool.tile([128, NBULKC], I16)
        nc.sync.dma_start(bulk_idx_sb[:], bulk_idx[:])
        uniq_idx_sb = cpool.tile([128, NT_U * NQ // 16], I16)
        nc.sync.dma_start(uniq_idx_sb[:], uniq_idx[:])

        uslots = NT_U * NQ // 128
        uniq_sb = cpool.tile([128, uslots, D], F32)
        ucols = UBLK // 16
        for ub in range(n_ublk):
            nc.gpsimd.dma_gather(
                out_ap=uniq_sb[:, ub * (UBLK // 128):(ub + 1) * (UBLK // 128), :],
                in_ap=u_shard[:],
                idxs_ap=uniq_idx_sb[:, ub * ucols:(ub + 1) * ucols],
                num_idxs=UBLK, num_idxs_reg=UBLK, elem_size=D,
                single_packet=False, queue_num=3)

        g_ps = None
        it_sb = None
        t_global = 0
        bulk_col = 0
        for w in range(n_iwin):
            i_win_ap = i_tab[w * IWIN:min(NI, (w + 1) * IWIN), :]
            sec_t = 0
            for blk in range(n_bulk_blocks[w]):
                nt_here = min(BLK_TILES, sec_pad[w] - sec_t)
                if nt_here <= 0:
                    break
                ibig = bulkpool.tile([128, BLK_TILES, D], F32, tag="ibig")
                nc.gpsimd.dma_gather(
                    out_ap=ibig[:], in_ap=i_win_ap,
                    idxs_ap=bulk_idx_sb[:, bulk_col:bulk_col + BLK_TILES * 8],
                    num_idxs=BLK_TILES * 128, num_idxs_reg=BLK_TILES * 128,
                    elem_size=D, single_packet=False, queue_num=blk % 4)
                bulk_col += BLK_TILES * 8

                for g0 in range(0, nt_here, CGRP):
                    gt = t_global + g0
                    us = gt * NQ // 128
                    ut_ps = pst.tile([64, 2, 128], F32, tag="tps")
                    nc.tensor.transpose(out=ut_ps[:, 0, :], in_=uniq_sb[:, us, :],
                                        identity=ident[:])
                    ut_sb = itpool.tile([64, 128], F32, tag="utsb")
                    nc.scalar.copy(ut_sb[:], ut_ps[:, 0, :])
                    c_ps = psc.tile([64, R, 128], F32, tag="cps")
                    for r in range(R):
                        nc.tensor.matmul(c_ps[:, r, :], lhsT=wb_sb[0:64, r, :],
                                         rhs=ut_sb[:], start=True, stop=False)
                        nc.tensor.matmul(c_ps[:, r, :], lhsT=b_sb[:, r, :],
                                         rhs=ones_sb[:], start=False, stop=True)
                    c_sb = csbpool.tile([64, R, 128], F32, tag="csb")
                    nc.vector.tensor_copy(c_sb[:], c_ps[:])

                    for tt in range(CGRP):
                        t = gt + tt
                        s = g0 + tt
                        if tt % 2 == 0:
                            it_ps = pst.tile([64, 2, 128], F32, tag="tps")
                            nc.tensor.transpose(out=it_ps[:, 0, :],
                                                in_=ibig[:, s, :],
                                                identity=ident[:])
                            nc.tensor.transpose(out=it_ps[:, 1, :],
                                                in_=ibig[:, s + 1, :],
                                                identity=ident[:])
                            it_sb = itpool.tile([64, 2, 128], F32, tag="itsb")
                            if (t // 2) % 2 == 0:
                                nc.vector.tensor_copy(it_sb[:], it_ps[:])
                            else:
                                nc.scalar.copy(it_sb[:], it_ps[:])
                        tb = t % TBATCH
                        if tb == 0:
                            g_ps = psg.tile([128, TBATCH, 128], F32, tag="gps")
                        nc.tensor.matmul(
                            g_ps[:, tb, 0:R * NQ].rearrange(
                                "p (r q) -> p r q", r=R),
                            lhsT=it_sb[:, tt % 2, :],
                            rhs=c_sb[:, :, NQ * tt:NQ * (tt + 1)],
                            start=True, stop=True)

                        if tb == TBATCH - 1:
                            t0 = t - TBATCH + 1
                            onehot = selpool.tile([128, TBATCH, NQ], F32,
                                                  tag="onehot")
                            nc.vector.tensor_tensor(
                                out=onehot[:],
                                in0=pos_sb[:, t0:t0 + TBATCH]
                                    .to_broadcast([128, TBATCH, NQ]),
                                in1=_bc_mid(iota_sb[:], TBATCH, 1),
                                op=mybir.AluOpType.is_equal)
                            gsel = selpool.tile([128, TBATCH, R, NQ], F32,
                                                tag="gsel")
                            g_view = bass.AP(
                                g_ps[:].tensor, g_ps[:].offset,
                                [g_ps[:].ap[0], (128, TBATCH), (NQ, R),
                                 (1, NQ)])
                            nc.vector.tensor_tensor(
                                out=gsel[:], in0=g_view,
                                in1=_bc_mid(onehot[:], R, 2),
                                op=mybir.AluOpType.mult)
                            ob = outpool.tile([128, TBATCH, R], F32, tag="ob")
                            nc.vector.tensor_reduce(
                                out=ob[:], in_=gsel[:],
                                axis=mybir.AxisListType.X,
                                op=mybir.AluOpType.add)
                            ot_ps = pst.tile([128, 128], F32, tag="tps")
                            nc.tensor.transpose(
                                out=ot_ps[0:TBATCH * R, :],
                                in_=ob[:].rearrange("p a b -> p (a b)"),
                                identity=ident[:])
                            ot_sb = outpool.tile([TBATCH * R, 128], F32,
                                                 tag="otsb")
                            nc.scalar.copy(ot_sb[:], ot_ps[0:TBATCH * R, :])
                            nc.scalar.dma_start(
                                out_dev[t0 * R:(t0 + TBATCH) * R, :], ot_sb[:])
                t_global += nt_here
                sec_t += nt_here

    nc.compile()
    return nc


# ----------------------------------------------------------------- kernel()

def _run(u_features, i_features, edge_user, edge_item, W, b, trace=False):
    u_features = np.asarray(u_features, dtype=np.float32)
    i_features = np.asarray(i_features, dtype=np.float32)
    W = np.asarray(W, dtype=np.float32)
    b = np.asarray(b, dtype=np.float32)

    in_maps, reasm, meta = host_prep(u_features, i_features,
                                     edge_user, edge_item)
    nc = build_program(meta)

    wb_host = np.zeros((65, R, D), dtype=np.float32)
    wb_host[:64] = np.transpose(W, (2, 0, 1))   # [k, r, j] = W[r, j, k]
    wb_host[64] = b                              # row 64: b[r, j]
    iota16 = np.tile(np.arange(NQ, dtype=np.float32), (128, 1))
    ones128 = np.ones((1, 128), dtype=np.float32)
    b_host = np.ascontiguousarray(b.reshape(1, R, D))
    for m in in_maps:
        m["wb"] = wb_host
        m["bvec"] = b_host
        m["iota16"] = iota16
        m["ones_in"] = ones128

    res = run_bass_kernel_spmd(nc, in_maps, list(range(N_CORES)), trace=trace)

    E, NT = meta["E"], meta["NT"]
    out = np.zeros((E, R), dtype=np.float32)
    for c in range(N_CORES):
        od = res.results[c]["out_dev"].reshape(NT, R, 128)
        se = reasm[c]                           # [NT, 128]
        valid = se >= 0
        out[se[valid]] = od.transpose(0, 2, 1)[valid]
    return out, res


def kernel(u_features, i_features, edge_user, edge_item, W, b):
    out, _ = _run(u_features, i_features, edge_user, edge_item, W, b)
    return out



# revision 21
# speedup vs baseline: 2.4078x; 2.4078x over previous
"""BiDecoder edge kernel for Trainium2, 8-core SPMD.

out[e, r] = (u[edge_user[e]] @ W[r].T + b[r]) . i[edge_item[e]]

Distribution: edges sharded by contiguous user ranges across 8 cores; each
core gets its own user-table slice plus a replicated item table.

Per-core algorithm (v3 — bf16 PE, 256B-padded tables, NQ=8):
  - host sorts the shard by (item_section, user) and packs 128-edge tiles
    with <= NQ unique users each; 16-tile C-groups hold 128 unique users;
    64-tile blocks drive the gather pipeline.
  - tables live in DRAM as bf16 rows padded to 256B; u-rows carry a 1.0 at
    column 64 so the bias rides the transpose as a 65th feature.
  - per block, gpsimd dma_gather (non-transpose: concurrent-queue safe)
    pulls per-edge item rows and per-group unique-user rows; trailing pad
    slots use idx=-1 so the ucode skips their descriptors.
  - PE transposes gathered rows (bf16), computes C[d',(r,uq)] = wb^T @
    UT_ext per C-group (bias via the ones lane) and G[e,(r,q)] = IT^T @
    C-slice per tile.
  - DVE selects q = pos(e) with a one-hot multiply + reduce; dense f32
    [128, NT*5] output, host scatters back to edge order.
"""
import numpy as np
import ml_dtypes
from contextlib import ExitStack

import concourse.bacc as bacc
import concourse.bass as bass
import concourse.tile as tile
from concourse import mybir
from concourse.bass_utils import run_bass_kernel_spmd
from concourse.masks import make_identity

F32 = mybir.dt.float32
BF16 = mybir.dt.bfloat16
I16 = mybir.dt.int16
BF = ml_dtypes.bfloat16

N_CORES = 8
D = 64
R = 5
NQ = 8             # max unique users per 128-edge tile
CGRP = 16          # tiles per C-group (CGRP*NQ = 128 unique rows)
TBATCH = 8         # tiles per select batch (one PSUM bank)
BLK_TILES = 64     # tiles per gather block (8192 edges, 4 C-groups)
IWIN = 32768       # item-section size (int16 index limit)


# ----------------------------------------------------------------- host prep

def _prep_core(users, items, eidx):
    """Tile-pack one core's edges, sorted by (item_section, user)."""
    iwin_all = items // IWIN
    order = np.lexsort((users, iwin_all))
    users = users[order]; items = items[order]; eidx = eidx[order]
    iwin_all = iwin_all[order]

    n = len(users)
    newgrp = np.ones(n, dtype=bool)
    if n > 1:
        newgrp[1:] = (users[1:] != users[:-1]) | (iwin_all[1:] != iwin_all[:-1])
    grp_starts = np.nonzero(newgrp)[0]
    grp_ends = np.append(grp_starts[1:], n)

    tile_win, slot_item, slot_eidx, slot_pos, tile_uniq = [], [], [], [], []
    ci, ce, cp, cuq = [], [], [], []
    cw = -1

    def flush():
        nonlocal ci, ce, cp, cuq
        if not ci:
            return
        pi = ci[0]
        while len(ci) < 128:
            ci.append(pi); ce.append(-1); cp.append(0)
        uq = list(cuq)
        while len(uq) < NQ:
            uq.append(uq[0])
        tile_win.append(cw)
        slot_item.append(ci); slot_eidx.append(ce)
        slot_pos.append(cp); tile_uniq.append(uq)
        ci, ce, cp, cuq = [], [], [], []

    for gs, ge in zip(grp_starts, grp_ends):
        w = int(iwin_all[gs]); u = int(users[gs])
        pos = int(gs)
        while pos < ge:
            if ci and (cw != w or len(ci) >= 128 or
                       (u not in cuq and len(cuq) >= NQ)):
                flush()
            cw = w
            if u not in cuq:
                cuq.append(u)
            p = cuq.index(u)
            take = min(int(ge) - pos, 128 - len(ci))
            ci.extend(items[pos:pos + take].tolist())
            ce.extend(eidx[pos:pos + take].tolist())
            cp.extend([p] * take)
            pos += take
    flush()

    return {
        "tile_win": np.array(tile_win, dtype=np.int64),
        "slot_item": np.array(slot_item, dtype=np.int64),
        "slot_eidx": np.array(slot_eidx, dtype=np.int64),
        "slot_pos": np.array(slot_pos, dtype=np.int64),
        "tile_uniq": np.array(tile_uniq, dtype=np.int64),
    }


def _wrap16(idx_i16):
    """[n] int16 (n%16==0) -> [128, n//16]: idx j -> row j%16, col j//16,
    16-row block replicated to 128 partitions."""
    n = len(idx_i16)
    w = idx_i16.reshape(n // 16, 16).T
    return np.ascontiguousarray(np.tile(w, (8, 1)))


def _padded_rows(rows_f32, pad_one=False):
    """[n, 64] f32 -> [n, 128] bf16 rows (256B), optional 1.0 at col 64."""
    n = rows_f32.shape[0]
    out = np.zeros((n, 128), dtype=BF)
    out[:, :64] = rows_f32.astype(BF)
    if pad_one:
        out[:, 64] = np.float32(1.0)
    return out


def host_prep(u_features, i_features, edge_user, edge_item):
    E = len(edge_user)
    NU = u_features.shape[0]
    NI = i_features.shape[0]
    n_iwin = (NI + IWIN - 1) // IWIN

    eu = np.asarray(edge_user, dtype=np.int64)
    ei = np.asarray(edge_item, dtype=np.int64)

    # shard by contiguous user ranges with ~equal edge counts
    order_u = np.argsort(eu, kind="stable")
    eus = eu[order_u]
    bounds = [0]
    for c in range(1, N_CORES):
        t = (E * c) // N_CORES
        while 0 < t < E and eus[t] == eus[t - 1]:
            t += 1
        bounds.append(min(t, E))
    bounds.append(E)

    cores = []
    for c in range(N_CORES):
        sl = order_u[bounds[c]:bounds[c + 1]]
        cores.append(_prep_core(eu[sl], ei[sl], sl.copy()))

    ushard_start, ushard_len = [], []
    for c in range(N_CORES):
        uq = cores[c]["tile_uniq"]
        lo = int(uq.min()) if uq.size else 0
        hi = int(uq.max()) + 1 if uq.size else 1
        ushard_start.append(lo); ushard_len.append(hi - lo)
    MAXU = max(ushard_len)
    assert MAXU <= 32768, f"user shard {MAXU} exceeds int16 range"

    # per-section tile counts, padded to BLK_TILES and equalized across cores
    sec_tiles = np.zeros((N_CORES, n_iwin), dtype=np.int64)
    for c in range(N_CORES):
        for w in range(n_iwin):
            sec_tiles[c, w] = int((cores[c]["tile_win"] == w).sum())
    sec_pad = ((sec_tiles.max(axis=0) + BLK_TILES - 1) // BLK_TILES) * BLK_TILES
    sec_pad = np.maximum(sec_pad, BLK_TILES)
    NT = int(sec_pad.sum())
    n_blocks = NT // BLK_TILES
    blk_sec = []
    for w in range(n_iwin):
        blk_sec += [w] * (int(sec_pad[w]) // BLK_TILES)

    i_pad = _padded_rows(np.asarray(i_features, dtype=np.float32))

    # pass 1: pack per-core tile arrays, find per-block valid counts
    packed = []
    nregs = np.full(n_blocks, 16, dtype=np.int64)
    for c in range(N_CORES):
        d = cores[c]
        slot_item_p = np.zeros((NT, 128), dtype=np.int64)
        slot_pos_p = np.zeros((NT, 128), dtype=np.int64)
        tile_uniq_p = np.zeros((NT, NQ), dtype=np.int64)
        slot_eidx_p = np.full((NT, 128), -1, dtype=np.int64)
        t_out = 0
        for w in range(n_iwin):
            sel = np.nonzero(d["tile_win"] == w)[0]
            k = len(sel); spw = int(sec_pad[w])
            if k:
                slot_item_p[t_out:t_out + k] = d["slot_item"][sel]
                slot_pos_p[t_out:t_out + k] = d["slot_pos"][sel]
                tile_uniq_p[t_out:t_out + k] = d["tile_uniq"][sel]
                slot_eidx_p[t_out:t_out + k] = d["slot_eidx"][sel]
            slot_item_p[t_out + k:t_out + spw] = w * IWIN
            tile_uniq_p[t_out + k:t_out + spw] = ushard_start[c]
            t_out += spw
        assert t_out == NT
        packed.append((slot_item_p, slot_pos_p, tile_uniq_p, slot_eidx_p))

        pad_flat = (slot_eidx_p < 0).reshape(n_blocks, BLK_TILES * 128)
        for b_ in range(n_blocks):
            p = pad_flat[b_]
            nv = int(np.nonzero(~p)[0][-1]) + 1 if not p.all() else 0
            nv16 = ((nv + 15) // 16) * 16
            nregs[b_] = max(nregs[b_], nv16)

    # pass 2: build per-core inputs with the GLOBAL trailing -1 cut so
    # count(idx>=0) matches num_idxs_reg on every core
    in_maps, reasm = [], []
    sec_base = np.array([blk_sec[t // BLK_TILES] * IWIN
                         for t in range(NT)], dtype=np.int64)
    for c in range(N_CORES):
        slot_item_p, slot_pos_p, tile_uniq_p, slot_eidx_p = packed[c]
        rel = (slot_item_p - sec_base[:, None]).astype(np.int16)
        rel_flat = rel.reshape(n_blocks, BLK_TILES * 128)
        for b_ in range(n_blocks):
            rel_flat[b_, int(nregs[b_]):] = -1
        bulk_wrapped = _wrap16(rel_flat.reshape(-1))

        uq = (tile_uniq_p - ushard_start[c]).astype(np.int16).ravel()
        uniq_wrapped = _wrap16(uq)

        pos_f32 = np.ascontiguousarray(slot_pos_p.T.astype(np.float32))

        ush = np.zeros((MAXU, D), dtype=np.float32)
        n_avail = min(MAXU, NU - ushard_start[c])
        ush[:n_avail] = u_features[ushard_start[c]:ushard_start[c] + n_avail]
        u_pad = _padded_rows(ush, pad_one=True)

        in_maps.append({
            "u_pad": u_pad,
            "bulk_idx": bulk_wrapped,
            "uniq_idx": uniq_wrapped,
            "pos": pos_f32,
        })
        reasm.append(slot_eidx_p)

    meta = {
        "NT": NT, "n_blocks": n_blocks, "nregs": [int(x) for x in nregs],
        "blk_sec": blk_sec,
        "n_iwin": n_iwin, "MAXU": MAXU, "NI": NI, "E": E,
    }
    return in_maps, i_pad, reasm, meta


# ------------------------------------------------------------ device program

def _bc_mid(ap, size, axis):
    """Insert a stride-0 dim of `size` at free-dim position `axis` (counting
    the partition dim as 0)."""
    lst = [list(x) for x in ap.ap]
    lst.insert(axis, [0, size])
    return bass.AP(ap.tensor, ap.offset, [tuple(x) for x in lst])


def build_program(meta, debug_dump=False):
    NT = meta["NT"]
    n_blocks = meta["n_blocks"]
    nregs = meta["nregs"]
    blk_sec = meta["blk_sec"]
    MAXU, NI = meta["MAXU"], meta["NI"]
    EPB = BLK_TILES * 128          # edges per block (8192)
    UPB = BLK_TILES * NQ           # unique users per block (512)
    NGRP = BLK_TILES // CGRP       # C-groups per block (4)

    nc = bacc.Bacc("TRN2", target_bir_lowering=False, debug=False,
                   num_devices=N_CORES, num_swdge_queues=4)

    i_tab = nc.dram_tensor("i_pad", [NI, 128], BF16, kind="ExternalInput").ap()
    u_tab = nc.dram_tensor("u_pad", [MAXU, 128], BF16, kind="ExternalInput").ap()
    bulk_in = nc.dram_tensor("bulk_idx", [128, NT * 8], I16,
                             kind="ExternalInput").ap()
    uniq_in = nc.dram_tensor("uniq_idx", [128, NT * NQ // 16], I16,
                             kind="ExternalInput").ap()
    pos_in = nc.dram_tensor("pos", [128, NT], F32, kind="ExternalInput").ap()
    wb_in = nc.dram_tensor("wb", [65, R, 64], BF16, kind="ExternalInput").ap()
    iota_in = nc.dram_tensor("iota8", [128, NQ], F32, kind="ExternalInput").ap()
    out_dev = nc.dram_tensor("out_dev", [128, NT * R], F32,
                             kind="ExternalOutput").ap()
    if debug_dump:
        dbg_ubig = nc.dram_tensor("dbg_ubig", [128, NGRP * 128], BF16,
                                  kind="ExternalOutput").ap()
        dbg_csb = nc.dram_tensor("dbg_csb", [64, R * 128], BF16,
                                 kind="ExternalOutput").ap()
        dbg_ibig = nc.dram_tensor("dbg_ibig", [128, 4 * 128], BF16,
                                  kind="ExternalOutput").ap()
        dbg_itsb = nc.dram_tensor("dbg_itsb", [64, 4 * 128], BF16,
                                  kind="ExternalOutput").ap()
        dbg_gsel = nc.dram_tensor("dbg_gsel", [128, TBATCH * R * NQ], BF16,
                                  kind="ExternalOutput").ap()

    with tile.TileContext(nc) as tc, ExitStack() as ctx:
        cpool = ctx.enter_context(tc.tile_pool(name="const", bufs=1))
        idxpool = ctx.enter_context(tc.tile_pool(name="idx", bufs=3))
        itpool = ctx.enter_context(tc.tile_pool(name="it", bufs=3))
        upool = ctx.enter_context(tc.tile_pool(name="uq", bufs=3))
        txpool = ctx.enter_context(tc.tile_pool(name="tx", bufs=4))
        csbpool = ctx.enter_context(tc.tile_pool(name="csb", bufs=2))
        ohpool = ctx.enter_context(tc.tile_pool(name="oh", bufs=2))
        selpool = ctx.enter_context(tc.tile_pool(name="sel", bufs=3))
        outpool = ctx.enter_context(tc.tile_pool(name="outb", bufs=2))
        pst = ctx.enter_context(tc.tile_pool(name="pst", bufs=2, space="PSUM"))
        psu = ctx.enter_context(tc.tile_pool(name="psu", bufs=1, space="PSUM"))
        psc = ctx.enter_context(tc.tile_pool(name="psc", bufs=1, space="PSUM"))
        psg = ctx.enter_context(tc.tile_pool(name="psg", bufs=2, space="PSUM"))

        ident = cpool.tile([128, 128], BF16)
        make_identity(nc, ident[:])
        wb_sb = cpool.tile([65, R, 64], BF16)
        nc.sync.dma_start(wb_sb[:], wb_in[:])
        iota_sb = cpool.tile([128, NQ], F32)
        nc.sync.dma_start(iota_sb[:], iota_in[:])
        pos_sb = cpool.tile([128, NT], F32)
        nc.sync.dma_start(pos_sb[:], pos_in[:])

        for blk in range(n_blocks):
            t0 = blk * BLK_TILES

            bidx = idxpool.tile([128, EPB // 16], I16, tag="bidx")
            nc.sync.dma_start(bidx[:], bulk_in[:, t0 * 8:(t0 + BLK_TILES) * 8])
            uidx = idxpool.tile([128, UPB // 16], I16, tag="uidx")
            nc.sync.dma_start(uidx[:],
                              uniq_in[:, t0 * NQ // 16:(t0 + BLK_TILES) * NQ // 16])

            ibig = itpool.tile([128, BLK_TILES, 128], BF16, tag="ibig")
            sec0 = blk_sec[blk] * IWIN
            nc.gpsimd.dma_gather(
                out_ap=ibig[:], in_ap=i_tab[sec0:NI, :], idxs_ap=bidx[:],
                num_idxs=EPB, num_idxs_reg=nregs[blk], elem_size=128,
                single_packet=False, queue_num=blk % 4)
            ubig = upool.tile([128, NGRP, 128], BF16, tag="ubig")
            nc.gpsimd.dma_gather(
                out_ap=ubig[:], in_ap=u_tab[:], idxs_ap=uidx[:],
                num_idxs=UPB, num_idxs_reg=UPB, elem_size=128,
                single_packet=False, queue_num=(blk + 2) % 4)

            onehot = ohpool.tile([128, BLK_TILES, NQ], F32, tag="oh")
            nc.vector.tensor_tensor(
                out=onehot[:],
                in0=pos_sb[:, t0:t0 + BLK_TILES]
                    .to_broadcast([128, BLK_TILES, NQ]),
                in1=_bc_mid(iota_sb[:], BLK_TILES, 1),
                op=mybir.AluOpType.is_equal)

            obuf = outpool.tile([128, BLK_TILES, R], F32, tag="ob")

            for g in range(NGRP):
                ut_ps = psu.tile([65, 128], BF16, tag="utps")
                nc.tensor.transpose(out=ut_ps[:], in_=ubig[:, g, 0:65],
                                    identity=ident[:])
                ut_sb = txpool.tile([65, 128], BF16, tag="utsb")
                nc.scalar.copy(ut_sb[:], ut_ps[:])

                c_ps = psc.tile([64, R, 128], F32, tag="cps")
                for r in range(R):
                    nc.tensor.matmul(c_ps[:, r, :], lhsT=wb_sb[0:65, r, :],
                                     rhs=ut_sb[:], start=True, stop=True)
                c_sb = csbpool.tile([64, R, 128], BF16, tag="csb")
                nc.scalar.copy(c_sb[:], c_ps[:])
                if debug_dump and blk == 0 and g == 0:
                    nc.sync.dma_start(
                        dbg_ubig[:], ubig[:].rearrange("p a b -> p (a b)"))
                    nc.sync.dma_start(
                        dbg_csb[:], c_sb[:].rearrange("p a b -> p (a b)"))
                    nc.sync.dma_start(
                        dbg_ibig[:],
                        ibig[:, 0:4, :].rearrange("p a b -> p (a b)"))

                g_ps = None
                it_sb = None
                for tt in range(CGRP):
                    s = g * CGRP + tt          # tile within block
                    if tt % 4 == 0:
                        it_ps = pst.tile([64, 4, 128], BF16, tag="itps")
                        for j in range(4):
                            nc.tensor.transpose(out=it_ps[:, j, :],
                                                in_=ibig[:, s + j, 0:64],
                                                identity=ident[:])
                        it_sb = txpool.tile([64, 4, 128], BF16, tag="itsb")
                        if (s // 4) % 2 == 0:
                            nc.vector.tensor_copy(it_sb[:], it_ps[:])
                        else:
                            nc.scalar.copy(it_sb[:], it_ps[:])
                    if debug_dump and blk == 0 and s == 0:
                        nc.sync.dma_start(
                            dbg_itsb[:],
                            it_sb[:].rearrange("p a b -> p (a b)"))
                    tb = s % TBATCH
                    if tb == 0:
                        g_ps = psg.tile([128, TBATCH, 64], F32, tag="gps")
                    nc.tensor.matmul(
                        g_ps[:, tb, 0:R * NQ].rearrange(
                            "p (r q) -> p r q", r=R),
                        lhsT=it_sb[:, tt % 4, :],
                        rhs=c_sb[:, :, NQ * tt:NQ * (tt + 1)],
                        start=True, stop=True)
                    if tb == TBATCH - 1:
                        s0 = s - TBATCH + 1
                        g_view = bass.AP(
                            g_ps[:].tensor, g_ps[:].offset,
                            [g_ps[:].ap[0], (64, TBATCH), (NQ, R), (1, NQ)])
                        gsel = selpool.tile([128, TBATCH, R, NQ], BF16,
                                            tag="gsel")
                        nc.vector.tensor_tensor(
                            out=gsel[:], in0=g_view,
                            in1=_bc_mid(onehot[:, s0:s0 + TBATCH, :], R, 2),
                            op=mybir.AluOpType.mult)
                        nc.vector.tensor_reduce(
                            out=obuf[:, s0:s0 + TBATCH, :], in_=gsel[:],
                            axis=mybir.AxisListType.X,
                            op=mybir.AluOpType.add)
                        if debug_dump and blk == 0 and s0 == 0:
                            nc.sync.dma_start(
                                dbg_gsel[:],
                                gsel[:].rearrange("p a b c -> p (a b c)"))

            nc.scalar.dma_start(
                out_dev[:, t0 * R:(t0 + BLK_TILES) * R],
                obuf[:].rearrange("p a b -> p (a b)"))

    nc.compile()
    return nc


# ----------------------------------------------------------------- kernel()

def _run(u_features, i_features, edge_user, edge_item, W, b, trace=False):
    u_features = np.asarray(u_features, dtype=np.float32)
    i_features = np.asarray(i_features, dtype=np.float32)
    W = np.asarray(W, dtype=np.float32)
    b = np.asarray(b, dtype=np.float32)

    in_maps, i_pad, reasm, meta = host_prep(
        u_features, i_features, edge_user, edge_item)
    nc = build_program(meta)

    wb_host = np.zeros((65, R, D), dtype=BF)
    wb_host[:64] = np.transpose(W, (2, 0, 1)).astype(BF)  # [k, r, j] = W[r,j,k]
    wb_host[64] = b.astype(BF)                            # row 64: b[r, j]
    iota8 = np.tile(np.arange(NQ, dtype=np.float32), (128, 1))
    for m in in_maps:
        m["i_pad"] = i_pad
        m["wb"] = wb_host
        m["iota8"] = iota8

    res = run_bass_kernel_spmd(nc, in_maps, list(range(N_CORES)), trace=trace)

    E, NT = meta["E"], meta["NT"]
    out = np.zeros((E, R), dtype=np.float32)
    for c in range(N_CORES):
        od = res.results[c]["out_dev"].reshape(128, NT, R)
        se = reasm[c]                           # [NT, 128]
        valid = se >= 0
        out[se[valid]] = od.transpose(1, 0, 2)[valid]
    return out, res


def kernel(u_features, i_features, edge_user, edge_item, W, b):
    out, _ = _run(u_features, i_features, edge_user, edge_item, W, b)
    return out


# revision 29
# speedup vs baseline: 3.1783x; 1.3200x over previous
"""BiDecoder edge kernel for Trainium2, 8-core SPMD.

out[e, r] = (u[edge_user[e]] @ W[r].T + b[r]) . i[edge_item[e]]

Distribution: edges sharded by contiguous user ranges across 8 cores; each
core gets its own user-table slice plus a replicated item table.

Per-core algorithm (v3 — bf16 PE, 256B-padded tables, NQ=8):
  - host sorts the shard by (item_section, user) and packs 128-edge tiles
    with <= NQ unique users each; 16-tile C-groups hold 128 unique users;
    64-tile blocks drive the gather pipeline.
  - tables live in DRAM as bf16 rows padded to 256B; u-rows carry a 1.0 at
    column 64 so the bias rides the transpose as a 65th feature.
  - per block, gpsimd dma_gather (non-transpose: concurrent-queue safe)
    pulls per-edge item rows and per-group unique-user rows; trailing pad
    slots use idx=-1 so the ucode skips their descriptors.
  - PE transposes gathered rows (bf16), computes C[d',(r,uq)] = wb^T @
    UT_ext per C-group (bias via the ones lane) and G[e,(r,q)] = IT^T @
    C-slice per tile.
  - DVE selects q = pos(e) with a one-hot multiply + reduce; dense f32
    [128, NT*5] output, host scatters back to edge order.
"""
import numpy as np
import ml_dtypes
from contextlib import ExitStack

import concourse.bacc as bacc
import concourse.bass as bass
import concourse.tile as tile
from concourse import mybir
from concourse.bass_utils import run_bass_kernel_spmd
from concourse.masks import make_identity

F32 = mybir.dt.float32
BF16 = mybir.dt.bfloat16
I16 = mybir.dt.int16
BF = ml_dtypes.bfloat16

N_CORES = 8
D = 64
R = 5
NQ = 16            # max unique users per 128-edge tile
CGRP = 8           # tiles per C-group (CGRP*NQ = 128 unique rows)
TBATCH = 8         # tiles per select batch
BLK_TILES = 64     # tiles per gather block (8192 edges, 8 C-groups)
IWIN = 32768       # item-section size (int16 index limit)


# ----------------------------------------------------------------- host prep

def _prep_core(users, items, eidx):
    """Tile-pack one core's edges, sorted by (item_section, user)."""
    iwin_all = items // IWIN
    order = np.lexsort((users, iwin_all))
    users = users[order]; items = items[order]; eidx = eidx[order]
    iwin_all = iwin_all[order]

    n = len(users)
    newgrp = np.ones(n, dtype=bool)
    if n > 1:
        newgrp[1:] = (users[1:] != users[:-1]) | (iwin_all[1:] != iwin_all[:-1])
    grp_starts = np.nonzero(newgrp)[0]
    grp_ends = np.append(grp_starts[1:], n)

    tile_win, slot_item, slot_eidx, slot_pos, tile_uniq = [], [], [], [], []
    ci, ce, cp, cuq = [], [], [], []
    cw = -1

    def flush():
        nonlocal ci, ce, cp, cuq
        if not ci:
            return
        pi = ci[0]
        while len(ci) < 128:
            ci.append(pi); ce.append(-1); cp.append(0)
        uq = list(cuq)
        while len(uq) < NQ:
            uq.append(uq[0])
        tile_win.append(cw)
        slot_item.append(ci); slot_eidx.append(ce)
        slot_pos.append(cp); tile_uniq.append(uq)
        ci, ce, cp, cuq = [], [], [], []

    for gs, ge in zip(grp_starts, grp_ends):
        w = int(iwin_all[gs]); u = int(users[gs])
        pos = int(gs)
        while pos < ge:
            if ci and (cw != w or len(ci) >= 128 or
                       (u not in cuq and len(cuq) >= NQ)):
                flush()
            cw = w
            if u not in cuq:
                cuq.append(u)
            p = cuq.index(u)
            take = min(int(ge) - pos, 128 - len(ci))
            ci.extend(items[pos:pos + take].tolist())
            ce.extend(eidx[pos:pos + take].tolist())
            cp.extend([p] * take)
            pos += take
    flush()

    return {
        "tile_win": np.array(tile_win, dtype=np.int64),
        "slot_item": np.array(slot_item, dtype=np.int64),
        "slot_eidx": np.array(slot_eidx, dtype=np.int64),
        "slot_pos": np.array(slot_pos, dtype=np.int64),
        "tile_uniq": np.array(tile_uniq, dtype=np.int64),
    }


def _wrap16(idx_i16):
    """[n] int16 (n%16==0) -> [128, n//16]: idx j -> row j%16, col j//16,
    16-row block replicated to 128 partitions."""
    n = len(idx_i16)
    w = idx_i16.reshape(n // 16, 16).T
    return np.ascontiguousarray(np.tile(w, (8, 1)))


def _padded_rows(rows_f32, pad_one=False):
    """[n, 64] f32 -> [n, 128] bf16 rows (256B), optional 1.0 at col 64."""
    n = rows_f32.shape[0]
    out = np.zeros((n, 128), dtype=BF)
    out[:, :64] = rows_f32.astype(BF)
    if pad_one:
        out[:, 64] = np.float32(1.0)
    return out


def host_prep(u_features, i_features, edge_user, edge_item):
    E = len(edge_user)
    NU = u_features.shape[0]
    NI = i_features.shape[0]
    n_iwin = (NI + IWIN - 1) // IWIN

    eu = np.asarray(edge_user, dtype=np.int64)
    ei = np.asarray(edge_item, dtype=np.int64)

    # shard by contiguous user ranges with ~equal edge counts
    order_u = np.argsort(eu, kind="stable")
    eus = eu[order_u]
    bounds = [0]
    for c in range(1, N_CORES):
        t = (E * c) // N_CORES
        while 0 < t < E and eus[t] == eus[t - 1]:
            t += 1
        bounds.append(min(t, E))
    bounds.append(E)

    cores = []
    for c in range(N_CORES):
        sl = order_u[bounds[c]:bounds[c + 1]]
        cores.append(_prep_core(eu[sl], ei[sl], sl.copy()))

    ushard_start, ushard_len = [], []
    for c in range(N_CORES):
        uq = cores[c]["tile_uniq"]
        lo = int(uq.min()) if uq.size else 0
        hi = int(uq.max()) + 1 if uq.size else 1
        ushard_start.append(lo); ushard_len.append(hi - lo)
    MAXU = max(ushard_len)
    assert MAXU <= 32768, f"user shard {MAXU} exceeds int16 range"

    # per-section tile counts, padded to BLK_TILES and equalized across cores
    sec_tiles = np.zeros((N_CORES, n_iwin), dtype=np.int64)
    for c in range(N_CORES):
        for w in range(n_iwin):
            sec_tiles[c, w] = int((cores[c]["tile_win"] == w).sum())
    sec_pad = ((sec_tiles.max(axis=0) + BLK_TILES - 1) // BLK_TILES) * BLK_TILES
    sec_pad = np.maximum(sec_pad, BLK_TILES)
    NT = int(sec_pad.sum())
    n_blocks = NT // BLK_TILES
    blk_sec = []
    for w in range(n_iwin):
        blk_sec += [w] * (int(sec_pad[w]) // BLK_TILES)

    i_pad = _padded_rows(np.asarray(i_features, dtype=np.float32))

    # pass 1: pack per-core tile arrays, find per-block valid counts
    packed = []
    nregs = np.full(n_blocks, 16, dtype=np.int64)
    for c in range(N_CORES):
        d = cores[c]
        slot_item_p = np.zeros((NT, 128), dtype=np.int64)
        slot_pos_p = np.zeros((NT, 128), dtype=np.int64)
        tile_uniq_p = np.zeros((NT, NQ), dtype=np.int64)
        slot_eidx_p = np.full((NT, 128), -1, dtype=np.int64)
        t_out = 0
        for w in range(n_iwin):
            sel = np.nonzero(d["tile_win"] == w)[0]
            k = len(sel); spw = int(sec_pad[w])
            if k:
                slot_item_p[t_out:t_out + k] = d["slot_item"][sel]
                slot_pos_p[t_out:t_out + k] = d["slot_pos"][sel]
                tile_uniq_p[t_out:t_out + k] = d["tile_uniq"][sel]
                slot_eidx_p[t_out:t_out + k] = d["slot_eidx"][sel]
            slot_item_p[t_out + k:t_out + spw] = w * IWIN
            tile_uniq_p[t_out + k:t_out + spw] = ushard_start[c]
            t_out += spw
        assert t_out == NT
        packed.append((slot_item_p, slot_pos_p, tile_uniq_p, slot_eidx_p))

        pad_flat = (slot_eidx_p < 0).reshape(n_blocks, BLK_TILES * 128)
        for b_ in range(n_blocks):
            p = pad_flat[b_]
            nv = int(np.nonzero(~p)[0][-1]) + 1 if not p.all() else 0
            nv16 = ((nv + 15) // 16) * 16
            nregs[b_] = max(nregs[b_], nv16)

    # pass 2: build per-core inputs with the GLOBAL trailing -1 cut so
    # count(idx>=0) matches num_idxs_reg on every core
    in_maps, reasm = [], []
    sec_base = np.array([blk_sec[t // BLK_TILES] * IWIN
                         for t in range(NT)], dtype=np.int64)
    for c in range(N_CORES):
        slot_item_p, slot_pos_p, tile_uniq_p, slot_eidx_p = packed[c]
        rel = (slot_item_p - sec_base[:, None]).astype(np.int16)
        rel_flat = rel.reshape(n_blocks, BLK_TILES * 128)
        for b_ in range(n_blocks):
            rel_flat[b_, int(nregs[b_]):] = -1
        bulk_wrapped = _wrap16(rel_flat.reshape(-1))

        uq = (tile_uniq_p - ushard_start[c]).astype(np.int16).ravel()
        uniq_wrapped = _wrap16(uq)

        pos_f32 = np.ascontiguousarray(slot_pos_p.T.astype(np.float32))

        ush = np.zeros((MAXU, D), dtype=np.float32)
        n_avail = min(MAXU, NU - ushard_start[c])
        ush[:n_avail] = u_features[ushard_start[c]:ushard_start[c] + n_avail]
        u_pad = _padded_rows(ush, pad_one=True)

        in_maps.append({
            "u_pad": u_pad,
            "bulk_idx": bulk_wrapped,
            "uniq_idx": uniq_wrapped,
            "pos": pos_f32,
        })
        reasm.append(slot_eidx_p)

    meta = {
        "NT": NT, "n_blocks": n_blocks, "nregs": [int(x) for x in nregs],
        "blk_sec": blk_sec,
        "n_iwin": n_iwin, "MAXU": MAXU, "NI": NI, "E": E,
    }
    return in_maps, i_pad, reasm, meta


# ------------------------------------------------------------ device program

def _bc_mid(ap, size, axis):
    """Insert a stride-0 dim of `size` at free-dim position `axis` (counting
    the partition dim as 0)."""
    lst = [list(x) for x in ap.ap]
    lst.insert(axis, [0, size])
    return bass.AP(ap.tensor, ap.offset, [tuple(x) for x in lst])


def build_program(meta, debug_dump=False):
    NT = meta["NT"]
    n_blocks = meta["n_blocks"]
    nregs = meta["nregs"]
    blk_sec = meta["blk_sec"]
    MAXU, NI = meta["MAXU"], meta["NI"]
    EPB = BLK_TILES * 128          # edges per block (8192)
    UPB = BLK_TILES * NQ           # unique users per block (512)
    NGRP = BLK_TILES // CGRP       # C-groups per block (4)

    nc = bacc.Bacc("TRN2", target_bir_lowering=False, debug=False,
                   num_devices=N_CORES, num_swdge_queues=4)

    i_tab = nc.dram_tensor("i_pad", [NI, 128], BF16, kind="ExternalInput").ap()
    u_tab = nc.dram_tensor("u_pad", [MAXU, 128], BF16, kind="ExternalInput").ap()
    bulk_in = nc.dram_tensor("bulk_idx", [128, NT * 8], I16,
                             kind="ExternalInput").ap()
    uniq_in = nc.dram_tensor("uniq_idx", [128, NT * NQ // 16], I16,
                             kind="ExternalInput").ap()
    pos_in = nc.dram_tensor("pos", [128, NT], F32, kind="ExternalInput").ap()
    wb_in = nc.dram_tensor("wb", [65, R, 64], BF16, kind="ExternalInput").ap()
    iota_in = nc.dram_tensor("iota8", [128, NQ], F32, kind="ExternalInput").ap()
    out_dev = nc.dram_tensor("out_dev", [128, NT * R], F32,
                             kind="ExternalOutput").ap()
    if debug_dump:
        dbg_ubig = nc.dram_tensor("dbg_ubig", [128, NGRP * 128], BF16,
                                  kind="ExternalOutput").ap()
        dbg_csb = nc.dram_tensor("dbg_csb", [64, R * 128], BF16,
                                 kind="ExternalOutput").ap()
        dbg_ibig = nc.dram_tensor("dbg_ibig", [128, 4 * 128], BF16,
                                  kind="ExternalOutput").ap()
        dbg_itsb = nc.dram_tensor("dbg_itsb", [64, 4 * 128], BF16,
                                  kind="ExternalOutput").ap()
        dbg_gsel = nc.dram_tensor("dbg_gsel", [128, TBATCH * R * NQ], BF16,
                                  kind="ExternalOutput").ap()

    with tile.TileContext(nc) as tc, ExitStack() as ctx:
        cpool = ctx.enter_context(tc.tile_pool(name="const", bufs=1))
        idxpool = ctx.enter_context(tc.tile_pool(name="idx", bufs=5))
        itpool = ctx.enter_context(tc.tile_pool(name="it", bufs=5))
        upool = ctx.enter_context(tc.tile_pool(name="uq", bufs=3))
        txpool = ctx.enter_context(tc.tile_pool(name="tx", bufs=3))
        csbpool = ctx.enter_context(tc.tile_pool(name="csb", bufs=2))
        ohpool = ctx.enter_context(tc.tile_pool(name="oh", bufs=2))
        selpool = ctx.enter_context(tc.tile_pool(name="sel", bufs=3))
        outpool = ctx.enter_context(tc.tile_pool(name="outb", bufs=2))
        pst = ctx.enter_context(tc.tile_pool(name="pst", bufs=1, space="PSUM"))
        psu = ctx.enter_context(tc.tile_pool(name="psu", bufs=1, space="PSUM"))
        psc = ctx.enter_context(tc.tile_pool(name="psc", bufs=1, space="PSUM"))
        psg = ctx.enter_context(tc.tile_pool(name="psg", bufs=2, space="PSUM"))

        ident = cpool.tile([128, 128], BF16)
        make_identity(nc, ident[:])
        wb_sb = cpool.tile([65, R, 64], BF16)
        nc.sync.dma_start(wb_sb[:], wb_in[:])
        iota_sb = cpool.tile([128, NQ], F32)
        nc.sync.dma_start(iota_sb[:], iota_in[:])
        pos_sb = cpool.tile([128, NT], F32)
        nc.sync.dma_start(pos_sb[:], pos_in[:])

        for blk in range(n_blocks):
            t0 = blk * BLK_TILES

            bidx = idxpool.tile([128, EPB // 16], I16, tag="bidx")
            nc.sync.dma_start(bidx[:], bulk_in[:, t0 * 8:(t0 + BLK_TILES) * 8])
            uidx = idxpool.tile([128, UPB // 16], I16, tag="uidx")
            nc.sync.dma_start(uidx[:],
                              uniq_in[:, t0 * NQ // 16:(t0 + BLK_TILES) * NQ // 16])

            ibig = itpool.tile([128, BLK_TILES, 128], BF16, tag="ibig")
            sec0 = blk_sec[blk] * IWIN
            nc.gpsimd.dma_gather(
                out_ap=ibig[:], in_ap=i_tab[sec0:NI, :], idxs_ap=bidx[:],
                num_idxs=EPB, num_idxs_reg=nregs[blk], elem_size=128,
                single_packet=False, queue_num=blk % 4)
            ubig = upool.tile([128, NGRP, 128], BF16, tag="ubig")
            nc.gpsimd.dma_gather(
                out_ap=ubig[:], in_ap=u_tab[:], idxs_ap=uidx[:],
                num_idxs=UPB, num_idxs_reg=UPB, elem_size=128,
                single_packet=False, queue_num=(blk + 2) % 4)

            onehot = ohpool.tile([128, BLK_TILES, NQ], F32, tag="oh")
            nc.vector.tensor_tensor(
                out=onehot[:],
                in0=pos_sb[:, t0:t0 + BLK_TILES]
                    .to_broadcast([128, BLK_TILES, NQ]),
                in1=_bc_mid(iota_sb[:], BLK_TILES, 1),
                op=mybir.AluOpType.is_equal)

            obuf = outpool.tile([128, BLK_TILES, R], F32, tag="ob")

            for g in range(NGRP):
                ut_ps = psu.tile([65, 128], BF16, tag="utps")
                nc.tensor.transpose(out=ut_ps[:], in_=ubig[:, g, 0:65],
                                    identity=ident[:])
                ut_sb = txpool.tile([65, 128], BF16, tag="utsb")
                nc.scalar.copy(ut_sb[:], ut_ps[:])

                c_ps = psc.tile([64, R, 128], F32, tag="cps")
                for r in range(R):
                    nc.tensor.matmul(c_ps[:, r, :], lhsT=wb_sb[0:65, r, :],
                                     rhs=ut_sb[:], start=True, stop=True)
                c_sb = csbpool.tile([64, R, 128], BF16, tag="csb")
                nc.scalar.copy(c_sb[:], c_ps[:])
                if debug_dump and blk == 0 and g == 0:
                    nc.sync.dma_start(
                        dbg_ubig[:], ubig[:].rearrange("p a b -> p (a b)"))
                    nc.sync.dma_start(
                        dbg_csb[:], c_sb[:].rearrange("p a b -> p (a b)"))
                    nc.sync.dma_start(
                        dbg_ibig[:],
                        ibig[:, 0:4, :].rearrange("p a b -> p (a b)"))

                g_ps = None
                it_sb = None
                for tt in range(CGRP):
                    s = g * CGRP + tt          # tile within block
                    if tt % 4 == 0:
                        # transpose the full padded row: partitions 0:64 of
                        # the result are the features, 64:128 pad (dropped
                        # at the copy)
                        it_ps = pst.tile([128, 4, 128], BF16, tag="itps")
                        for j in range(4):
                            nc.tensor.transpose(out=it_ps[:, j, :],
                                                in_=ibig[:, s + j, :],
                                                identity=ident[:])
                        it_sb = txpool.tile([64, 4, 128], BF16, tag="itsb")
                        if (s // 4) % 2 == 0:
                            nc.vector.tensor_copy(it_sb[:], it_ps[0:64, :, :])
                        else:
                            nc.scalar.copy(it_sb[:], it_ps[0:64, :, :])
                    if debug_dump and blk == 0 and s == 0:
                        nc.sync.dma_start(
                            dbg_itsb[:],
                            it_sb[:].rearrange("p a b -> p (a b)"))
                    tb = s % TBATCH
                    if tb == 0:
                        g_ps = psg.tile([128, TBATCH, 128], F32, tag="gps")
                    nc.tensor.matmul(
                        g_ps[:, tb, 0:R * NQ].rearrange(
                            "p (r q) -> p r q", r=R),
                        lhsT=it_sb[:, tt % 4, :],
                        rhs=c_sb[:, :, NQ * tt:NQ * (tt + 1)],
                        start=True, stop=True)
                    if tb == TBATCH - 1:
                        s0 = s - TBATCH + 1
                        g_view = bass.AP(
                            g_ps[:].tensor, g_ps[:].offset,
                            [g_ps[:].ap[0], (128, TBATCH), (NQ, R), (1, NQ)])
                        gsel = selpool.tile([128, TBATCH, R, NQ], BF16,
                                            tag="gsel")
                        nc.vector.tensor_tensor(
                            out=gsel[:], in0=g_view,
                            in1=_bc_mid(onehot[:, s0:s0 + TBATCH, :], R, 2),
                            op=mybir.AluOpType.mult)
                        nc.vector.tensor_reduce(
                            out=obuf[:, s0:s0 + TBATCH, :], in_=gsel[:],
                            axis=mybir.AxisListType.X,
                            op=mybir.AluOpType.add)
                        if debug_dump and blk == 0 and s0 == 0:
                            nc.sync.dma_start(
                                dbg_gsel[:],
                                gsel[:].rearrange("p a b c -> p (a b c)"))

            nc.scalar.dma_start(
                out_dev[:, t0 * R:(t0 + BLK_TILES) * R],
                obuf[:].rearrange("p a b -> p (a b)"))

    nc.compile()
    return nc


# ----------------------------------------------------------------- kernel()

def _run(u_features, i_features, edge_user, edge_item, W, b, trace=False):
    u_features = np.asarray(u_features, dtype=np.float32)
    i_features = np.asarray(i_features, dtype=np.float32)
    W = np.asarray(W, dtype=np.float32)
    b = np.asarray(b, dtype=np.float32)

    in_maps, i_pad, reasm, meta = host_prep(
        u_features, i_features, edge_user, edge_item)
    nc = build_program(meta)

    wb_host = np.zeros((65, R, D), dtype=BF)
    wb_host[:64] = np.transpose(W, (2, 0, 1)).astype(BF)  # [k, r, j] = W[r,j,k]
    wb_host[64] = b.astype(BF)                            # row 64: b[r, j]
    iota8 = np.tile(np.arange(NQ, dtype=np.float32), (128, 1))
    for m in in_maps:
        m["i_pad"] = i_pad
        m["wb"] = wb_host
        m["iota8"] = iota8

    res = run_bass_kernel_spmd(nc, in_maps, list(range(N_CORES)), trace=trace)

    E, NT = meta["E"], meta["NT"]
    out = np.zeros((E, R), dtype=np.float32)
    for c in range(N_CORES):
        od = res.results[c]["out_dev"].reshape(128, NT, R)
        se = reasm[c]                           # [NT, 128]
        valid = se >= 0
        out[se[valid]] = od.transpose(1, 0, 2)[valid]
    return out, res


def kernel(u_features, i_features, edge_user, edge_item, W, b):
    out, _ = _run(u_features, i_features, edge_user, edge_item, W, b)
    return out


# revision 35
# speedup vs baseline: 3.6289x; 1.1418x over previous
"""BiDecoder edge kernel for Trainium2, 8-core SPMD.

out[e, r] = (u[edge_user[e]] @ W[r].T + b[r]) . i[edge_item[e]]

Distribution: edges sharded by contiguous user ranges across 8 cores; each
core gets its own user-table slice plus a replicated item table.

Per-core algorithm (v3 — bf16 PE, 256B-padded tables, NQ=8):
  - host sorts the shard by (item_section, user) and packs 128-edge tiles
    with <= NQ unique users each; 16-tile C-groups hold 128 unique users;
    64-tile blocks drive the gather pipeline.
  - tables live in DRAM as bf16 rows padded to 256B; u-rows carry a 1.0 at
    column 64 so the bias rides the transpose as a 65th feature.
  - per block, gpsimd dma_gather (non-transpose: concurrent-queue safe)
    pulls per-edge item rows and per-group unique-user rows; trailing pad
    slots use idx=-1 so the ucode skips their descriptors.
  - PE transposes gathered rows (bf16), computes C[d',(r,uq)] = wb^T @
    UT_ext per C-group (bias via the ones lane) and G[e,(r,q)] = IT^T @
    C-slice per tile.
  - DVE selects q = pos(e) with a one-hot multiply + reduce; dense f32
    [128, NT*5] output, host scatters back to edge order.
"""
import numpy as np
import ml_dtypes
from contextlib import ExitStack

import concourse.bacc as bacc
import concourse.bass as bass
import concourse.tile as tile
from concourse import mybir
from concourse.bass_utils import run_bass_kernel_spmd
from concourse.masks import make_identity

F32 = mybir.dt.float32
BF16 = mybir.dt.bfloat16
I16 = mybir.dt.int16
BF = ml_dtypes.bfloat16

N_CORES = 8
D = 64
R = 5
NQ = 16            # max unique users per 128-edge tile
CGRP = 8           # tiles per C-group (CGRP*NQ = 128 unique rows)
TBATCH = 8         # tiles per select batch
BLK_TILES = 64     # tiles per gather block (8192 edges, 8 C-groups)
CHUNK = 4096       # idx per item-gather chunk (32 tiles)
IWIN = 32768       # item-section size (int16 index limit)


# ----------------------------------------------------------------- host prep

def _prep_core(users, items, eidx):
    """Tile-pack one core's edges, sorted by (item_section, user)."""
    iwin_all = items // IWIN
    order = np.lexsort((users, iwin_all))
    users = users[order]; items = items[order]; eidx = eidx[order]
    iwin_all = iwin_all[order]

    n = len(users)
    newgrp = np.ones(n, dtype=bool)
    if n > 1:
        newgrp[1:] = (users[1:] != users[:-1]) | (iwin_all[1:] != iwin_all[:-1])
    grp_starts = np.nonzero(newgrp)[0]
    grp_ends = np.append(grp_starts[1:], n)

    tile_win, slot_item, slot_eidx, slot_pos, tile_uniq = [], [], [], [], []
    ci, ce, cp, cuq = [], [], [], []
    cw = -1

    def flush():
        nonlocal ci, ce, cp, cuq
        if not ci:
            return
        pi = ci[0]
        while len(ci) < 128:
            ci.append(pi); ce.append(-1); cp.append(0)
        uq = list(cuq)
        while len(uq) < NQ:
            uq.append(uq[0])
        tile_win.append(cw)
        slot_item.append(ci); slot_eidx.append(ce)
        slot_pos.append(cp); tile_uniq.append(uq)
        ci, ce, cp, cuq = [], [], [], []

    for gs, ge in zip(grp_starts, grp_ends):
        w = int(iwin_all[gs]); u = int(users[gs])
        pos = int(gs)
        while pos < ge:
            if ci and (cw != w or len(ci) >= 128 or
                       (u not in cuq and len(cuq) >= NQ)):
                flush()
            cw = w
            if u not in cuq:
                cuq.append(u)
            p = cuq.index(u)
            take = min(int(ge) - pos, 128 - len(ci))
            ci.extend(items[pos:pos + take].tolist())
            ce.extend(eidx[pos:pos + take].tolist())
            cp.extend([p] * take)
            pos += take
    flush()

    return {
        "tile_win": np.array(tile_win, dtype=np.int64),
        "slot_item": np.array(slot_item, dtype=np.int64),
        "slot_eidx": np.array(slot_eidx, dtype=np.int64),
        "slot_pos": np.array(slot_pos, dtype=np.int64),
        "tile_uniq": np.array(tile_uniq, dtype=np.int64),
    }


def _wrap16(idx_i16):
    """[n] int16 (n%16==0) -> [128, n//16]: idx j -> row j%16, col j//16,
    16-row block replicated to 128 partitions."""
    n = len(idx_i16)
    w = idx_i16.reshape(n // 16, 16).T
    return np.ascontiguousarray(np.tile(w, (8, 1)))


def _padded_rows(rows_f32, pad_one=False):
    """[n, 64] f32 -> [n, 128] bf16 rows (256B), optional 1.0 at col 64."""
    n = rows_f32.shape[0]
    out = np.zeros((n, 128), dtype=BF)
    out[:, :64] = rows_f32.astype(BF)
    if pad_one:
        out[:, 64] = np.float32(1.0)
    return out


def host_prep(u_features, i_features, edge_user, edge_item):
    E = len(edge_user)
    NU = u_features.shape[0]
    NI = i_features.shape[0]
    n_iwin = (NI + IWIN - 1) // IWIN

    eu = np.asarray(edge_user, dtype=np.int64)
    ei = np.asarray(edge_item, dtype=np.int64)

    # shard by contiguous user ranges with ~equal edge counts
    order_u = np.argsort(eu, kind="stable")
    eus = eu[order_u]
    bounds = [0]
    for c in range(1, N_CORES):
        t = (E * c) // N_CORES
        while 0 < t < E and eus[t] == eus[t - 1]:
            t += 1
        bounds.append(min(t, E))
    bounds.append(E)

    cores = []
    for c in range(N_CORES):
        sl = order_u[bounds[c]:bounds[c + 1]]
        cores.append(_prep_core(eu[sl], ei[sl], sl.copy()))

    ushard_start, ushard_len = [], []
    for c in range(N_CORES):
        uq = cores[c]["tile_uniq"]
        lo = int(uq.min()) if uq.size else 0
        hi = int(uq.max()) + 1 if uq.size else 1
        ushard_start.append(lo); ushard_len.append(hi - lo)
    MAXU = max(ushard_len)
    assert MAXU <= 32768, f"user shard {MAXU} exceeds int16 range"

    # per-section tile counts, padded to BLK_TILES and equalized across cores
    sec_tiles = np.zeros((N_CORES, n_iwin), dtype=np.int64)
    for c in range(N_CORES):
        for w in range(n_iwin):
            sec_tiles[c, w] = int((cores[c]["tile_win"] == w).sum())
    sec_pad = ((sec_tiles.max(axis=0) + BLK_TILES - 1) // BLK_TILES) * BLK_TILES
    sec_pad = np.maximum(sec_pad, BLK_TILES)
    NT = int(sec_pad.sum())
    n_blocks = NT // BLK_TILES
    blk_sec = []
    for w in range(n_iwin):
        blk_sec += [w] * (int(sec_pad[w]) // BLK_TILES)

    i_pad = _padded_rows(np.asarray(i_features, dtype=np.float32))

    # pass 1: pack per-core tile arrays, find per-chunk valid counts
    packed = []
    n_chunks = NT * 128 // CHUNK
    nregs = np.full(n_chunks, 16, dtype=np.int64)
    for c in range(N_CORES):
        d = cores[c]
        slot_item_p = np.zeros((NT, 128), dtype=np.int64)
        slot_pos_p = np.zeros((NT, 128), dtype=np.int64)
        tile_uniq_p = np.zeros((NT, NQ), dtype=np.int64)
        slot_eidx_p = np.full((NT, 128), -1, dtype=np.int64)
        t_out = 0
        for w in range(n_iwin):
            sel = np.nonzero(d["tile_win"] == w)[0]
            k = len(sel); spw = int(sec_pad[w])
            if k:
                slot_item_p[t_out:t_out + k] = d["slot_item"][sel]
                slot_pos_p[t_out:t_out + k] = d["slot_pos"][sel]
                tile_uniq_p[t_out:t_out + k] = d["tile_uniq"][sel]
                slot_eidx_p[t_out:t_out + k] = d["slot_eidx"][sel]
            slot_item_p[t_out + k:t_out + spw] = w * IWIN
            tile_uniq_p[t_out + k:t_out + spw] = ushard_start[c]
            t_out += spw
        assert t_out == NT
        packed.append((slot_item_p, slot_pos_p, tile_uniq_p, slot_eidx_p))

        pad_flat = (slot_eidx_p < 0).reshape(-1, CHUNK)
        for b_ in range(pad_flat.shape[0]):
            p = pad_flat[b_]
            nv = int(np.nonzero(~p)[0][-1]) + 1 if not p.all() else 0
            nv16 = ((nv + 15) // 16) * 16
            nregs[b_] = max(nregs[b_], nv16)

    # pass 2: build per-core inputs with the GLOBAL trailing -1 cut so
    # count(idx>=0) matches num_idxs_reg on every core
    in_maps, reasm = [], []
    sec_base = np.array([blk_sec[t // BLK_TILES] * IWIN
                         for t in range(NT)], dtype=np.int64)
    for c in range(N_CORES):
        slot_item_p, slot_pos_p, tile_uniq_p, slot_eidx_p = packed[c]
        rel = (slot_item_p - sec_base[:, None]).astype(np.int16)
        rel_flat = rel.reshape(-1, CHUNK)
        for b_ in range(rel_flat.shape[0]):
            rel_flat[b_, int(nregs[b_]):] = -1
        bulk_wrapped = _wrap16(rel_flat.reshape(-1))

        uq = (tile_uniq_p - ushard_start[c]).astype(np.int16).ravel()
        uniq_wrapped = _wrap16(uq)

        pos_f32 = np.ascontiguousarray(slot_pos_p.T.astype(np.float32))

        ush = np.zeros((MAXU, D), dtype=np.float32)
        n_avail = min(MAXU, NU - ushard_start[c])
        ush[:n_avail] = u_features[ushard_start[c]:ushard_start[c] + n_avail]
        u_pad = _padded_rows(ush, pad_one=True)

        in_maps.append({
            "u_pad": u_pad,
            "bulk_idx": bulk_wrapped,
            "uniq_idx": uniq_wrapped,
            "pos": pos_f32,
        })
        reasm.append(slot_eidx_p)

    meta = {
        "NT": NT, "n_blocks": n_blocks, "nregs": [int(x) for x in nregs],
        "blk_sec": blk_sec,
        "n_iwin": n_iwin, "MAXU": MAXU, "NI": NI, "E": E,
    }
    return in_maps, i_pad, reasm, meta


# ------------------------------------------------------------ device program

def _bc_mid(ap, size, axis):
    """Insert a stride-0 dim of `size` at free-dim position `axis` (counting
    the partition dim as 0)."""
    lst = [list(x) for x in ap.ap]
    lst.insert(axis, [0, size])
    return bass.AP(ap.tensor, ap.offset, [tuple(x) for x in lst])


def build_program(meta, debug_dump=False):
    NT = meta["NT"]
    n_blocks = meta["n_blocks"]
    nregs = meta["nregs"]
    blk_sec = meta["blk_sec"]
    MAXU, NI = meta["MAXU"], meta["NI"]
    EPB = BLK_TILES * 128          # edges per block (8192)
    UPB = BLK_TILES * NQ           # unique users per block (512)
    NGRP = BLK_TILES // CGRP       # C-groups per block (4)

    nc = bacc.Bacc("TRN2", target_bir_lowering=False, debug=False,
                   num_devices=N_CORES, num_swdge_queues=4)

    i_tab = nc.dram_tensor("i_pad", [NI, 128], BF16, kind="ExternalInput").ap()
    u_tab = nc.dram_tensor("u_pad", [MAXU, 128], BF16, kind="ExternalInput").ap()
    bulk_in = nc.dram_tensor("bulk_idx", [128, NT * 8], I16,
                             kind="ExternalInput").ap()
    uniq_in = nc.dram_tensor("uniq_idx", [128, NT * NQ // 16], I16,
                             kind="ExternalInput").ap()
    pos_in = nc.dram_tensor("pos", [128, NT], F32, kind="ExternalInput").ap()
    wb_in = nc.dram_tensor("wb", [65, R, 64], BF16, kind="ExternalInput").ap()
    iota_in = nc.dram_tensor("iota8", [128, NQ], F32, kind="ExternalInput").ap()
    out_dev = nc.dram_tensor("out_dev", [128, NT * R], F32,
                             kind="ExternalOutput").ap()
    if debug_dump:
        dbg_ubig = nc.dram_tensor("dbg_ubig", [128, NGRP * 128], BF16,
                                  kind="ExternalOutput").ap()
        dbg_csb = nc.dram_tensor("dbg_csb", [64, R * 128], BF16,
                                 kind="ExternalOutput").ap()
        dbg_ibig = nc.dram_tensor("dbg_ibig", [128, 4 * 128], BF16,
                                  kind="ExternalOutput").ap()
        dbg_itsb = nc.dram_tensor("dbg_itsb", [64, 4 * 128], BF16,
                                  kind="ExternalOutput").ap()
        dbg_gsel = nc.dram_tensor("dbg_gsel", [128, TBATCH * R * NQ], BF16,
                                  kind="ExternalOutput").ap()

    with tile.TileContext(nc) as tc, ExitStack() as ctx:
        cpool = ctx.enter_context(tc.tile_pool(name="const", bufs=1))
        idxpool = ctx.enter_context(tc.tile_pool(name="idx", bufs=5))
        itpool = ctx.enter_context(tc.tile_pool(name="it", bufs=5))
        upool = ctx.enter_context(tc.tile_pool(name="uq", bufs=3))
        txpool = ctx.enter_context(tc.tile_pool(name="tx", bufs=3))
        csbpool = ctx.enter_context(tc.tile_pool(name="csb", bufs=2))
        ohpool = ctx.enter_context(tc.tile_pool(name="oh", bufs=2))
        selpool = ctx.enter_context(tc.tile_pool(name="sel", bufs=3))
        outpool = ctx.enter_context(tc.tile_pool(name="outb", bufs=2))
        pst = ctx.enter_context(tc.tile_pool(name="pst", bufs=1, space="PSUM"))
        psu = ctx.enter_context(tc.tile_pool(name="psu", bufs=1, space="PSUM"))
        psc = ctx.enter_context(tc.tile_pool(name="psc", bufs=1, space="PSUM"))
        psg = ctx.enter_context(tc.tile_pool(name="psg", bufs=2, space="PSUM"))

        ident = cpool.tile([128, 128], BF16)
        make_identity(nc, ident[:])
        wb_sb = cpool.tile([65, R, 64], BF16)
        nc.sync.dma_start(wb_sb[:], wb_in[:])
        iota_sb = cpool.tile([128, NQ], F32)
        nc.sync.dma_start(iota_sb[:], iota_in[:])
        pos_sb = cpool.tile([128, NT], F32)
        nc.sync.dma_start(pos_sb[:], pos_in[:])

        gq = [0]
        for blk in range(n_blocks):
            t0 = blk * BLK_TILES

            bidx = idxpool.tile([128, EPB // 16], I16, tag="bidx")
            nc.sync.dma_start(bidx[:], bulk_in[:, t0 * 8:(t0 + BLK_TILES) * 8])
            uidx = idxpool.tile([128, UPB // 16], I16, tag="uidx")
            nc.sync.dma_start(uidx[:],
                              uniq_in[:, t0 * NQ // 16:(t0 + BLK_TILES) * NQ // 16])

            ibig = itpool.tile([128, BLK_TILES, 128], BF16, tag="ibig")
            sec0 = blk_sec[blk] * IWIN
            ctiles = CHUNK // 128
            for ch in range(EPB // CHUNK):
                nc.gpsimd.dma_gather(
                    out_ap=ibig[:, ch * ctiles:(ch + 1) * ctiles, :],
                    in_ap=i_tab[sec0:NI, :],
                    idxs_ap=bidx[:, ch * CHUNK // 16:(ch + 1) * CHUNK // 16],
                    num_idxs=CHUNK,
                    num_idxs_reg=nregs[blk * (EPB // CHUNK) + ch],
                    elem_size=128,
                    single_packet=False, queue_num=gq[0] % 4)
                gq[0] += 1
            ubig = upool.tile([128, NGRP, 128], BF16, tag="ubig")
            nc.gpsimd.dma_gather(
                out_ap=ubig[:], in_ap=u_tab[:], idxs_ap=uidx[:],
                num_idxs=UPB, num_idxs_reg=UPB, elem_size=128,
                single_packet=False, queue_num=gq[0] % 4)
            gq[0] += 1

            onehot = ohpool.tile([128, BLK_TILES, NQ], F32, tag="oh")
            nc.vector.tensor_tensor(
                out=onehot[:],
                in0=pos_sb[:, t0:t0 + BLK_TILES]
                    .to_broadcast([128, BLK_TILES, NQ]),
                in1=_bc_mid(iota_sb[:], BLK_TILES, 1),
                op=mybir.AluOpType.is_equal)

            obuf = outpool.tile([128, BLK_TILES, R], F32, tag="ob")

            for g in range(NGRP):
                ut_ps = psu.tile([65, 128], BF16, tag="utps")
                nc.tensor.transpose(out=ut_ps[:], in_=ubig[:, g, 0:65],
                                    identity=ident[:])
                ut_sb = txpool.tile([65, 128], BF16, tag="utsb")
                nc.scalar.copy(ut_sb[:], ut_ps[:])

                c_ps = psc.tile([64, R, 128], F32, tag="cps")
                for r in range(R):
                    nc.tensor.matmul(c_ps[:, r, :], lhsT=wb_sb[0:65, r, :],
                                     rhs=ut_sb[:], start=True, stop=True)
                c_sb = csbpool.tile([64, R, 128], BF16, tag="csb")
                nc.scalar.copy(c_sb[:], c_ps[:])
                if debug_dump and blk == 0 and g == 0:
                    nc.sync.dma_start(
                        dbg_ubig[:], ubig[:].rearrange("p a b -> p (a b)"))
                    nc.sync.dma_start(
                        dbg_csb[:], c_sb[:].rearrange("p a b -> p (a b)"))
                    nc.sync.dma_start(
                        dbg_ibig[:],
                        ibig[:, 0:4, :].rearrange("p a b -> p (a b)"))

                g_ps = None
                it_sb = None
                for tt in range(CGRP):
                    s = g * CGRP + tt          # tile within block
                    if tt % 4 == 0:
                        # transpose the full padded row: partitions 0:64 of
                        # the result are the features, 64:128 pad (dropped
                        # at the copy)
                        it_ps = pst.tile([128, 4, 128], BF16, tag="itps")
                        for j in range(4):
                            nc.tensor.transpose(out=it_ps[:, j, :],
                                                in_=ibig[:, s + j, :],
                                                identity=ident[:])
                        it_sb = txpool.tile([64, 4, 128], BF16, tag="itsb")
                        if (s // 4) % 2 == 0:
                            nc.vector.tensor_copy(it_sb[:], it_ps[0:64, :, :])
                        else:
                            nc.scalar.copy(it_sb[:], it_ps[0:64, :, :])
                    if debug_dump and blk == 0 and s == 0:
                        nc.sync.dma_start(
                            dbg_itsb[:],
                            it_sb[:].rearrange("p a b -> p (a b)"))
                    tb = s % TBATCH
                    if tb == 0:
                        g_ps = psg.tile([128, TBATCH, 128], F32, tag="gps")
                    nc.tensor.matmul(
                        g_ps[:, tb, 0:R * NQ].rearrange(
                            "p (r q) -> p r q", r=R),
                        lhsT=it_sb[:, tt % 4, :],
                        rhs=c_sb[:, :, NQ * tt:NQ * (tt + 1)],
                        start=True, stop=True)
                    if tb == TBATCH - 1:
                        s0 = s - TBATCH + 1
                        g_view = bass.AP(
                            g_ps[:].tensor, g_ps[:].offset,
                            [g_ps[:].ap[0], (128, TBATCH), (NQ, R), (1, NQ)])
                        gsel = selpool.tile([128, TBATCH, R, NQ], BF16,
                                            tag="gsel")
                        nc.vector.tensor_tensor(
                            out=gsel[:], in0=g_view,
                            in1=_bc_mid(onehot[:, s0:s0 + TBATCH, :], R, 2),
                            op=mybir.AluOpType.mult)
                        nc.vector.tensor_reduce(
                            out=obuf[:, s0:s0 + TBATCH, :], in_=gsel[:],
                            axis=mybir.AxisListType.X,
                            op=mybir.AluOpType.add)
                        if debug_dump and blk == 0 and s0 == 0:
                            nc.sync.dma_start(
                                dbg_gsel[:],
                                gsel[:].rearrange("p a b c -> p (a b c)"))

            nc.scalar.dma_start(
                out_dev[:, t0 * R:(t0 + BLK_TILES) * R],
                obuf[:].rearrange("p a b -> p (a b)"))

    nc.compile()
    return nc


# ----------------------------------------------------------------- kernel()

def _run(u_features, i_features, edge_user, edge_item, W, b, trace=False):
    u_features = np.asarray(u_features, dtype=np.float32)
    i_features = np.asarray(i_features, dtype=np.float32)
    W = np.asarray(W, dtype=np.float32)
    b = np.asarray(b, dtype=np.float32)

    in_maps, i_pad, reasm, meta = host_prep(
        u_features, i_features, edge_user, edge_item)
    nc = build_program(meta)

    wb_host = np.zeros((65, R, D), dtype=BF)
    wb_host[:64] = np.transpose(W, (2, 0, 1)).astype(BF)  # [k, r, j] = W[r,j,k]
    wb_host[64] = b.astype(BF)                            # row 64: b[r, j]
    iota8 = np.tile(np.arange(NQ, dtype=np.float32), (128, 1))
    for m in in_maps:
        m["i_pad"] = i_pad
        m["wb"] = wb_host
        m["iota8"] = iota8

    res = run_bass_kernel_spmd(nc, in_maps, list(range(N_CORES)), trace=trace)

    E, NT = meta["E"], meta["NT"]
    out = np.zeros((E, R), dtype=np.float32)
    for c in range(N_CORES):
        od = res.results[c]["out_dev"].reshape(128, NT, R)
        se = reasm[c]                           # [NT, 128]
        valid = se >= 0
        out[se[valid]] = od.transpose(1, 0, 2)[valid]
    return out, res


def kernel(u_features, i_features, edge_user, edge_item, W, b):
    out, _ = _run(u_features, i_features, edge_user, edge_item, W, b)
    return out
